# revision 11
# baseline (speedup 1.0000x reference)
"""GCN (2-layer, PyG GCNConv-style) on 8 Trainium2 NeuronCores.

Sharding: nodes row-sharded 8 ways by dst (12500/core, padded to 12544).
Edges (incl. self-loops) bucketed to the core owning dst, laid out as
[substream(4)][window(98)][B slots] where substream = (src-table-half,
src-parity) so each dma_gather call reads one sub-table with int16 local
indices, and window = 128 consecutive dst nodes.

Per edge slot the device gathers (a) the message row h'[src] from the
pair-packed bf16 table (256B granularity) and (b) a one-hot row
I[dst%128] from a constant identity table; a matmul S^T @ msg then
accumulates the segment sum for each 128-node window in PSUM.  Norm is
separable: h' = (x@W1)*dinv at the src owner, final scale by dinv at the
dst owner, so no per-edge norm is needed.  h' is replicated across cores
with an AllGather between phases.  Layer 2 reuses the same edge
structure; W2/b2 are folded in after aggregation, then log_softmax.
"""

import numpy as np
import ml_dtypes

import sys

sys.path.insert(0, "/opt/trn_rl_repo")

import concourse.bass as bass
import concourse.bacc as bacc
import concourse.tile as tile
import concourse.mybir as mybir
from concourse.bass_utils import run_bass_kernel_spmd

BF16 = ml_dtypes.bfloat16

NC = 8
N_NODES = 100000
NPC = N_NODES // NC          # 12500 real nodes per core
W = 98                       # windows per core
NPCP = W * 128               # 12544 padded nodes per core
KT = 4                       # k-tiles for x@W1 (500 -> 512)
F_IN_P = KT * 128
H = 64                       # hidden dim
C = 5                        # classes
LO_PAIRS = 32000             # pair-rows in the lo sub-table
TOT_PAIRS = NC * NPCP // 2   # 50176


def _wrap_idx(flat):
    """[n] int16 -> [128, n/16] wrapped-by-16, replicated to 128 partitions."""
    n = flat.shape[0]
    w = flat.reshape(n // 16, 16).T  # [16, n/16]
    return np.ascontiguousarray(np.tile(w, (8, 1)))


def _host_prep(x, edge_index, W1, b1, W2, b2):
    src = edge_index[0].astype(np.int64)
    dst = edge_index[1].astype(np.int64)
    loops = np.arange(N_NODES, dtype=np.int64)
    src = np.concatenate([src, loops])
    dst = np.concatenate([dst, loops])

    core = dst // NPC
    dl = dst % NPC                      # core-local dst
    tsrc = NPCP * (src // NPC) + (src % NPC)   # padded-table node id of src
    pair = tsrc // 2
    sub = (pair >= LO_PAIRS).astype(np.int64) * 2 + (tsrc % 2)
    win = dl // 128
    dloc = dl % 128

    # per-(core, sub, win) counts -> slot budget B
    gkey = (core * 4 + sub) * W + win
    cnt = np.bincount(gkey, minlength=NC * 4 * W)
    B = max(640, int(np.ceil(cnt.max() / 128)) * 128)
    CW = max(1, min(8192 // B, 6))   # windows per chunk (PSUM: 6 window banks + 2 aux)
    chunks = [CW] * (W // CW) + ([W % CW] if W % CW else [])

    TOT = 4 * W * B                     # slots per core
    order = np.argsort(gkey, kind="stable")
    starts = np.zeros(NC * 4 * W + 1, dtype=np.int64)
    np.cumsum(cnt, out=starts[1:])
    within = np.arange(gkey.shape[0], dtype=np.int64) - starts[gkey[order]]
    # slot position inside the edge's own core block
    k = gkey[order] % (4 * W)
    slotpos = k * B + within

    midx = np.zeros((NC, TOT), dtype=np.int16)
    sidx = np.full((NC, TOT), 128, dtype=np.int16)  # 128 -> zero row of I-table
    oc = core[order]
    localized_pair = pair[order] - np.where(sub[order] >= 2, LO_PAIRS, 0)
    midx[oc, slotpos] = localized_pair.astype(np.int16)
    sidx[oc, slotpos] = dloc[order].astype(np.int16)

    # row_ptr per core (real degree incl self-loops; pad nodes get deg 1)
    rps = []
    for c in range(NC):
        cnt_nodes = np.bincount(dl[core == c], minlength=NPCP).astype(np.int32)
        cnt_nodes[NPC:] = 1
        rp = np.zeros(NPCP + 1, dtype=np.int32)
        np.cumsum(cnt_nodes, out=rp[1:])
        rps.append(rp)

    # dense inputs
    x = np.asarray(x)
    W1 = np.asarray(W1)
    b1 = np.asarray(b1)
    W2 = np.asarray(W2)
    b2 = np.asarray(b2)
    W1b = np.zeros((F_IN_P, H), dtype=BF16)
    W1b[:500] = W1.astype(BF16)
    W2b = W2.astype(BF16)
    b1rep = np.tile(b1.astype(np.float32)[None, :], (128, 1))
    b2rep = np.tile(b2.astype(np.float32)[None, :], (128, 1))

    in_maps = []
    for c in range(NC):
        xT = np.zeros((F_IN_P, NPCP), dtype=BF16)
        xT[:500, :NPC] = np.ascontiguousarray(x[c * NPC:(c + 1) * NPC].T).astype(BF16)
        in_maps.append({
            "xT": xT,
            "W1b": W1b,
            "W2b": W2b,
            "b1rep": b1rep,
            "b2rep": b2rep,
            "rp": rps[c],
            "midx": _wrap_idx(midx[c]),
            "sidx": _wrap_idx(sidx[c]),
        })
    return in_maps, B, chunks


def _build(B, chunks):
    TOT = 4 * W * B
    nc = bacc.Bacc("TRN2", target_bir_lowering=False, debug=False, num_devices=NC)

    xT = nc.dram_tensor("xT", [F_IN_P, NPCP], mybir.dt.bfloat16, kind="ExternalInput")
    W1b = nc.dram_tensor("W1b", [F_IN_P, H], mybir.dt.bfloat16, kind="ExternalInput")
    W2b = nc.dram_tensor("W2b", [H, C], mybir.dt.bfloat16, kind="ExternalInput")
    b1rep = nc.dram_tensor("b1rep", [128, H], mybir.dt.float32, kind="ExternalInput")
    b2rep = nc.dram_tensor("b2rep", [128, C], mybir.dt.float32, kind="ExternalInput")
    rp = nc.dram_tensor("rp", [NPCP + 1], mybir.dt.int32, kind="ExternalInput")
    midx = nc.dram_tensor("midx", [128, TOT // 16], mybir.dt.int16, kind="ExternalInput")
    sidx = nc.dram_tensor("sidx", [128, TOT // 16], mybir.dt.int16, kind="ExternalInput")

    out = nc.dram_tensor("out", [NPCP, C], mybir.dt.float32, kind="ExternalOutput")

    itab_np = np.zeros((256, 128), dtype=BF16)
    itab_np[:128] = np.eye(128, dtype=np.float32).astype(BF16)
    itab = nc.inline_tensor(itab_np, name="itab")
    eye_np = np.eye(128, dtype=np.float32).astype(BF16)
    eye_d = nc.inline_tensor(eye_np, name="eye128")

    ag_in1 = nc.dram_tensor("ag_in1", [NPCP, H], mybir.dt.bfloat16)
    ag_in2 = nc.dram_tensor("ag_in2", [NPCP, H], mybir.dt.bfloat16)
    table1 = nc.dram_tensor("table1", [TOT_PAIRS, 128], mybir.dt.bfloat16, addr_space="Shared")
    table2 = nc.dram_tensor("table2", [TOT_PAIRS, 128], mybir.dt.bfloat16, addr_space="Shared")

    rg = [list(range(NC))]

    with tile.TileContext(nc) as tc:
        import contextlib
        with contextlib.ExitStack() as ctx:
            persist = ctx.enter_context(tc.tile_pool(name="persist", bufs=1))
            # constants
            b1t = persist.tile([128, H], mybir.dt.float32)
            nc.sync.dma_start(b1t[:], b1rep[:])
            b2t = persist.tile([128, C], mybir.dt.float32)
            nc.sync.dma_start(b2t[:], b2rep[:])
            w2t = persist.tile([H, C], mybir.dt.bfloat16)
            nc.sync.dma_start(w2t[:], W2b[:])
            eyet = persist.tile([128, 128], mybir.dt.bfloat16)
            nc.sync.dma_start(eyet[:], eye_d[:])

            # dinv = rsqrt(diff(rp)) laid out [128 part, W]
            rpa = persist.tile([128, W], mybir.dt.int32)
            nc.sync.dma_start(rpa[:], rp[0:NPCP].rearrange("(w p) -> p w", p=128))
            rpb = persist.tile([128, W], mybir.dt.int32)
            nc.sync.dma_start(rpb[:], rp[1:NPCP + 1].rearrange("(w p) -> p w", p=128))
            deg = persist.tile([128, W], mybir.dt.float32)
            nc.vector.tensor_tensor(out=deg[:], in0=rpb[:], in1=rpa[:], op=mybir.AluOpType.subtract)
            dsq = persist.tile([128, W], mybir.dt.float32)
            nc.scalar.activation(dsq[:], deg[:], mybir.ActivationFunctionType.Sqrt)
            dinv = persist.tile([128, W], mybir.dt.float32)
            nc.vector.reciprocal(dinv[:], dsq[:])

            # ---------------- phase 1: h' = (x @ W1) * dinv ----------------
            hshard = persist.tile([128, W, H], mybir.dt.bfloat16)
            with tc.tile_pool(name="xtp", bufs=KT) as xtp, \
                 tc.tile_pool(name="w1p", bufs=KT) as w1p, \
                 tc.tile_pool(name="ps1", bufs=4, space="PSUM") as ps1:
                w1tiles = []
                xtiles = []
                for kt in range(KT):
                    w1k = w1p.tile([128, H], mybir.dt.bfloat16, tag="w1k")
                    nc.sync.dma_start(w1k[:], W1b[kt * 128:(kt + 1) * 128, :])
                    w1tiles.append(w1k)
                    xk = xtp.tile([128, NPCP], mybir.dt.bfloat16, tag="xk")
                    nc.sync.dma_start(xk[:], xT[kt * 128:(kt + 1) * 128, :])
                    xtiles.append(xk)
                for w in range(W):
                    ps = ps1.tile([128, H], mybir.dt.float32, space="PSUM", tag="psh")
                    for kt in range(KT):
                        nc.tensor.matmul(
                            ps[:], lhsT=xtiles[kt][:, w * 128:(w + 1) * 128],
                            rhs=w1tiles[kt][:], start=(kt == 0), stop=(kt == KT - 1),
                        )
                    nc.vector.tensor_scalar(
                        out=hshard[:, w, :], in0=ps, scalar1=dinv[:, w:w + 1],
                        scalar2=None, op0=mybir.AluOpType.mult,
                    )
            nc.sync.dma_start(ag_in1[:].rearrange("(w p) f -> p w f", p=128), hshard[:])
            nc.gpsimd.collective_compute(
                "AllGather", mybir.AluOpType.bypass, replica_groups=rg,
                ins=[ag_in1[:]],
                outs=[table1[:]],
            )

            # idx tiles (shared by both layers); pool opened after phase-1
            # pools free their SBUF
            idxp = ctx.enter_context(tc.tile_pool(name="idxp", bufs=1))
            mit = idxp.tile([128, TOT // 16], mybir.dt.int16)
            nc.sync.dma_start(mit[:], midx[:])
            sit = idxp.tile([128, TOT // 16], mybir.dt.int16)
            nc.sync.dma_start(sit[:], sidx[:])

            h1shard = persist.tile([128, W, H], mybir.dt.bfloat16)
            outc = persist.tile([128, W, C], mybir.dt.float32)

            def message_pass(layer, table):
                """layer 1 -> writes h1shard; layer 2 -> writes outc."""
                with tc.tile_pool(name=f"mt{layer}", bufs=2) as mtp, \
                     tc.tile_pool(name=f"st{layer}", bufs=2) as stp, \
                     tc.tile_pool(name=f"ps{layer}g", bufs=1, space="PSUM") as psg, \
                     tc.tile_pool(name=f"ps{layer}x", bufs=1, space="PSUM") as psx, \
                     tc.tile_pool(name=f"fin{layer}", bufs=4) as finp:
                    w0 = 0
                    for cw in chunks:
                        nidx = cw * B
                        cols = nidx // 128
                        pstiles = [
                            psg.tile([128, H], mybir.dt.float32, space="PSUM",
                                     tag=f"psg{wi}", name=f"psg_{layer}_{w0}_{wi}")
                            for wi in range(cw)
                        ]
                        for sub in range(4):
                            base = (sub * W + w0) * B
                            mt = mtp.tile([128, cols, 128], mybir.dt.bfloat16, tag="mt")
                            src_view = table[0:LO_PAIRS, :] if sub < 2 else table[LO_PAIRS:TOT_PAIRS, :]
                            nc.gpsimd.dma_gather(
                                mt[:], src_view, mit[:, base // 16:(base + nidx) // 16],
                                nidx, nidx, 128, single_packet=False,
                            )
                            st = stp.tile([128, cols, 128], mybir.dt.bfloat16, tag="st")
                            nc.gpsimd.dma_gather(
                                st[:], itab[:, :], sit[:, base // 16:(base + nidx) // 16],
                                nidx, nidx, 128, single_packet=False,
                            )
                            po = 0 if (sub % 2 == 0) else H
                            for wi in range(cw):
                                for j in range(B // 128):
                                    col = wi * (B // 128) + j
                                    nc.tensor.matmul(
                                        pstiles[wi][:],
                                        lhsT=st[:, col, :],
                                        rhs=mt[:, col, po:po + H],
                                        start=(sub == 0 and j == 0),
                                        stop=(sub == 3 and j == (B // 128) - 1),
                                    )
                        for wi in range(cw):
                            w = w0 + wi
                            ps = pstiles[wi][:]
                            if layer == 1:
                                t1 = finp.tile([128, H], mybir.dt.float32, tag="t1")
                                nc.vector.tensor_scalar(
                                    out=t1[:], in0=ps, scalar1=dinv[:, w:w + 1],
                                    scalar2=None, op0=mybir.AluOpType.mult,
                                )
                                t2 = finp.tile([128, H], mybir.dt.float32, tag="t2")
                                nc.vector.tensor_tensor(
                                    out=t2[:], in0=t1[:], in1=b1t[:], op=mybir.AluOpType.add)
                                t3 = finp.tile([128, H], mybir.dt.float32, tag="t3")
                                nc.scalar.activation(t3[:], t2[:], mybir.ActivationFunctionType.Relu)
                                nc.vector.tensor_scalar(
                                    out=h1shard[:, w, :], in0=t3[:], scalar1=dinv[:, w:w + 1],
                                    scalar2=None, op0=mybir.AluOpType.mult,
                                )
                            else:
                                gt = finp.tile([128, H], mybir.dt.bfloat16, tag="gt")
                                nc.vector.tensor_scalar(
                                    out=gt[:], in0=ps, scalar1=dinv[:, w:w + 1],
                                    scalar2=None, op0=mybir.AluOpType.mult,
                                )
                                trp = psx.tile([H, 128], mybir.dt.bfloat16, space="PSUM", tag="trp")
                                nc.tensor.transpose(out=trp[:], in_=gt[:], identity=eyet[:])
                                trs = finp.tile([H, 128], mybir.dt.bfloat16, tag="trs")
                                nc.vector.tensor_copy(out=trs[:], in_=trp[:])
                                op = psx.tile([128, C], mybir.dt.float32, space="PSUM", tag="op")
                                nc.tensor.matmul(op[:], lhsT=trs[:], rhs=w2t[:], start=True, stop=True)
                                # += b2, then log_softmax over C
                                xw = finp.tile([128, C], mybir.dt.float32, tag="xw")
                                nc.vector.tensor_tensor(out=xw[:], in0=op[:], in1=b2t[:], op=mybir.AluOpType.add)
                                m = finp.tile([128, 1], mybir.dt.float32, tag="m")
                                nc.vector.reduce_max(m[:], xw[:], axis=mybir.AxisListType.X)
                                negm = finp.tile([128, 1], mybir.dt.float32, tag="negm")
                                nc.vector.tensor_scalar(
                                    out=negm[:], in0=m[:], scalar1=-1.0, scalar2=None,
                                    op0=mybir.AluOpType.mult)
                                e = finp.tile([128, C], mybir.dt.float32, tag="e")
                                nc.scalar.activation(
                                    e[:], xw[:], mybir.ActivationFunctionType.Exp, bias=negm[:])
                                s = finp.tile([128, 1], mybir.dt.float32, tag="s")
                                nc.vector.reduce_sum(s[:], e[:], axis=mybir.AxisListType.X)
                                ls = finp.tile([128, 1], mybir.dt.float32, tag="ls")
                                nc.scalar.activation(ls[:], s[:], mybir.ActivationFunctionType.Ln)
                                tot = finp.tile([128, 1], mybir.dt.float32, tag="tot")
                                nc.vector.tensor_tensor(out=tot[:], in0=m[:], in1=ls[:], op=mybir.AluOpType.add)
                                nc.vector.tensor_scalar(
                                    out=outc[:, w, :], in0=xw[:], scalar1=tot[:],
                                    scalar2=None, op0=mybir.AluOpType.subtract)
                        w0 += cw

            message_pass(1, table1)
            nc.sync.dma_start(ag_in2[:].rearrange("(w p) f -> p w f", p=128), h1shard[:])
            nc.gpsimd.collective_compute(
                "AllGather", mybir.AluOpType.bypass, replica_groups=rg,
                ins=[ag_in2[:]], outs=[table2[:]],
            )
            message_pass(2, table2)
            nc.sync.dma_start(out[:].rearrange("(w p) f -> p w f", p=128), outc[:])

    nc.compile()
    return nc


_CACHE = {}


def kernel(x, edge_index, W1, b1, W2, b2):
    in_maps, B, chunks = _host_prep(x, edge_index, W1, b1, W2, b2)
    key = (B, tuple(chunks))
    if key not in _CACHE:
        _CACHE[key] = _build(B, chunks)
    nc = _CACHE[key]
    res = run_bass_kernel_spmd(nc, in_maps, core_ids=list(range(NC)))
    shards = [res.results[c]["out"][:NPC] for c in range(NC)]
    return np.concatenate(shards, axis=0).astype(np.float32)
